# revision 31
# baseline (speedup 1.0000x reference)
"""Trainium2 Bass kernel for nn_AwareDecoder segment first/last gather.

Problem: input [16, 2048, 1024] f32, number_mask [16, 2048] int64 with ids in
[0, 512]. For each segment id i in [0, 512): find first/last row-major token
position with that id, gather those rows of the flattened input, concat ->
out [512, 2048] f32.

Strategy (8 NeuronCores, segment-sharded - no collectives):
  core c owns segments [64c, 64c+64). Host passes ids narrowed to int16 and
  localized to the core's segment range (out-of-range -> -1). Each core:
    - loads the 64KB id tile split over both HWDGE queues and the gpsimd
      software queue (per-queue descriptor issue rate is the bottleneck),
    - gpsimd local_scatter writes (global token pos + 1) into a per-partition
      [128, 64] segment table (ids within a 256-token partition row are
      unique, absent segments stay 0); the min side lifts absent entries to
      65535 in the left half of the same [128, 128] tile,
    - one cast + one PE transpose put segments on partitions; reduce_min on
      rows 0:64 and reduce_max on rows 64:128 + a -1 decode yield the 128
      gather indices,
    - one indirect DMA gathers the 64 first + 64 last rows (512KB of the
      128MB input) in a single continuous stream, the out slice streams on
      both HWDGE queues.
Host concatenates the 8 slices.
"""
import numpy as np

import concourse.bass as bass
import concourse.tile as tile
from concourse import bacc, mybir
from concourse import bass_utils
from concourse.masks import make_identity

P = 128            # partitions
L = 32768          # B*S tokens
H = 1024           # hidden
NSEG = 512         # segments
NCORES = 8
SEG_PER_CORE = NSEG // NCORES            # 64
TOK_PER_PART = L // P                    # 256 tokens per partition
F32 = mybir.dt.float32
I32 = mybir.dt.int32
I16 = mybir.dt.int16
U16 = mybir.dt.uint16


def build_nc():
    nc = bacc.Bacc("TRN2", target_bir_lowering=False, debug=False)

    x = nc.dram_tensor("x", [L, H], F32, kind="ExternalInput")
    # per-core localized ids: value in [0, 64) for own segments, -1 otherwise
    ids_in = nc.dram_tensor("ids16", [P, TOK_PER_PART], I16, kind="ExternalInput")
    out = nc.dram_tensor("out", [SEG_PER_CORE, 2 * H], F32, kind="ExternalOutput")

    with tile.TileContext(nc) as tc:
        with tc.tile_pool(name="sb", bufs=1) as sb, \
             tc.tile_pool(name="ps", bufs=1, space="PSUM") as ps:

            # ---- id tile in: HWDGE queues only (software queue is ~16x
            # slower per descriptor)
            ids_t = sb.tile([P, TOK_PER_PART], I16)
            nc.sync.dma_start(ids_t[0:64, :], ids_in.ap()[0:64, :])
            nc.scalar.dma_start(ids_t[64:128, :], ids_in.ap()[64:128, :])

            # ---- constants generated on-chip while the id DMA is in flight
            data = sb.tile([P, TOK_PER_PART], U16)
            nc.gpsimd.iota(data[:], pattern=[[1, TOK_PER_PART]], base=1,
                           channel_multiplier=TOK_PER_PART)
            ident = sb.tile([P, P], F32)
            make_identity(nc, ident[:])
            # warm the gpsimd software DMA queue during the ids-semaphore
            # wait so the critical gather doesn't pay the cold-start ramp
            warm = sb.tile([1, TOK_PER_PART], I16)
            nc.gpsimd.dma_start(warm[0:1, :], ids_in.ap()[0:1, :])

            # ---- scatter: table[p, s] = global pos + 1 of s's occurrence in
            # partition p (0 if absent). Lands in the right half of M.
            M = sb.tile([P, P], U16)
            nc.gpsimd.local_scatter(
                out_ap=M[:, SEG_PER_CORE:P], data_ap=data[:], idxs_ap=ids_t[:],
                channels=P, num_elems=SEG_PER_CORE, num_idxs=TOK_PER_PART)

            # ---- min-side encoding in the left half: table, absent -> 65535
            lift = sb.tile([P, SEG_PER_CORE], U16)
            nc.vector.tensor_scalar(lift[:], M[:, SEG_PER_CORE:P], 0, 65535,
                                    op0=mybir.AluOpType.is_equal,
                                    op1=mybir.AluOpType.mult)
            nc.vector.tensor_tensor(out=M[:, 0:SEG_PER_CORE],
                                    in0=M[:, SEG_PER_CORE:P], in1=lift[:],
                                    op=mybir.AluOpType.add)

            # ---- one transpose so segments sit on partitions
            Mf = sb.tile([P, P], F32)
            nc.vector.tensor_copy(Mf[:], M[:])
            T = ps.tile([P, P], F32)
            nc.tensor.transpose(out=T[:], in_=Mf[:], identity=ident[:])

            # rows 0..63: min of lifted -> first+1; 64..127: max -> last+1
            enc = sb.tile([P, 1], F32)
            nc.vector.tensor_reduce(enc[0:SEG_PER_CORE, :], T[0:SEG_PER_CORE, :],
                                    axis=mybir.AxisListType.X,
                                    op=mybir.AluOpType.min)
            nc.vector.tensor_reduce(enc[SEG_PER_CORE:P, :], T[SEG_PER_CORE:P, :],
                                    axis=mybir.AxisListType.X,
                                    op=mybir.AluOpType.max)
            idx_i = sb.tile([P, 1], I32)
            nc.vector.tensor_scalar_add(idx_i[:], enc[:], -1.0)

            # ---- one continuous 128-row gather, outs on both HWDGE queues
            rows = sb.tile([P, H], F32)
            nc.gpsimd.indirect_dma_start(
                out=rows[:], out_offset=None, in_=x.ap(),
                in_offset=bass.IndirectOffsetOnAxis(ap=idx_i[:, 0:1], axis=0))
            nc.sync.dma_start(out.ap()[:, 0:H], rows[0:SEG_PER_CORE, :])
            nc.scalar.dma_start(out.ap()[:, H:2 * H], rows[SEG_PER_CORE:P, :])

    nc.compile()
    return nc


_NC = None


def _get_nc():
    global _NC
    if _NC is None:
        _NC = build_nc()
    return _NC


def make_in_maps(input, number_mask):
    x = np.ascontiguousarray(np.asarray(input), dtype=np.float32).reshape(L, H)
    nm = np.asarray(number_mask).reshape(L).astype(np.int16)
    in_maps = []
    for c in range(NCORES):
        loc = (nm - SEG_PER_CORE * c).astype(np.int16)
        loc[(loc < 0) | (loc >= SEG_PER_CORE)] = -1
        in_maps.append({"x": x, "ids16": loc.reshape(P, TOK_PER_PART)})
    return in_maps


def kernel(input, number_mask, n, concat, **_):
    assert int(n) == NSEG and int(concat) == 1
    nc = _get_nc()
    in_maps = make_in_maps(input, number_mask)
    res = bass_utils.run_bass_kernel_spmd(nc, in_maps, core_ids=list(range(NCORES)))
    return np.concatenate([res.results[c]["out"] for c in range(NCORES)], axis=0)
